# revision 1
# baseline (speedup 1.0000x reference)
"""Distributed Trainium2 kernel for APA iterative sparse propagation.

Scheme (8 NeuronCores, SPMD):
  - Destination nodes partitioned across cores (6272 per core = 49 windows x 128).
  - Node-feature table [50240, 128] bf16 lives in DRAM, replicated on every
    core via per-iteration AllGather of the per-core shard. Table rows hold
    x_tilde = dinv[n] * out[n] (source-side symmetric normalization folded in),
    plus one extras row per rank carrying the shard's column sum (for the mean
    term) - so the mean needs no extra collective.
  - Per window (128 dests): dma_gather the source rows for its edges
    (parity-split even/odd table views so gather indices fit int16), then a
    chain of PE matmuls with a resident fp8 0/1 indicator matrix S performs the
    segment sum into PSUM. Epilogue: scale by alpha*dinv*a (dest-side norm +
    known-reset coefficient), add the rank-1 mean term (PE outer product) and
    the known-reset constant, emit the new shard (bf16, rescaled by dinv) to
    the AllGather staging buffer.
"""

import os
import numpy as np
import ml_dtypes

import concourse.bass as bass
import concourse.bacc as bacc
import concourse.mybir as mybir
import concourse.tile as tile
from concourse.bass_utils import run_bass_kernel_spmd

bf16 = ml_dtypes.bfloat16
f8e4 = ml_dtypes.float8_e4m3

N, E, DF = 50000, 800000, 100
NCORES = 8
DPC = 6272            # dests per core
NW = DPC // 128       # 49 windows
RR = DPC + 8          # table rows per rank (extras row at local 6272)
TROWS = RR * NCORES   # 50240
HT = TROWS // 2       # 25120 row-pairs
N_ITER = int(os.environ.get("APA_NITER", "30"))


# ----------------------------------------------------------------- host prep
def host_prep(x, edge_index, known_idx, alpha, beta):
    x = np.asarray(x, np.float32)
    row = np.asarray(edge_index[0], np.int64)
    col = np.asarray(edge_index[1], np.int64)
    known_idx = np.asarray(known_idx, np.int64)
    alpha = float(alpha)
    beta = float(beta)

    deg = np.bincount(row, minlength=N).astype(np.float32)
    dinv = np.where(deg > 0, 1.0 / np.sqrt(np.maximum(deg, 1.0)), 0.0).astype(np.float32)

    m = np.zeros(N, np.float32)
    m[known_idx] = 1.0
    out0 = np.zeros((N, DF), np.float32)
    out0[known_idx] = x[known_idx]
    a = (1.0 - m * (1.0 - beta)).astype(np.float32)
    binit = (m * (1.0 - beta))[:, None] * out0

    core = row // DPC
    local = row - core * DPC
    wi = local // 128
    slot_d = local % 128
    tr = RR * (col // DPC) + (col % DPC)
    par = (tr % 2).astype(np.int64)
    hidx = (tr // 2).astype(np.int64)
    assert hidx.max() < 32768

    counts = np.zeros((NCORES, NW, 2), np.int64)
    np.add.at(counts, (core, wi, par), 1)
    T_e = int(np.ceil(counts[:, :, 0].max() / 128))
    T_o = int(np.ceil(counts[:, :, 1].max() / 128))
    TT = T_e + T_o

    # rank edges within (core, window, parity) groups
    order = np.lexsort((par, wi, core))
    inv = np.empty_like(order)
    inv[order] = np.arange(E)
    grp = (core * NW + wi) * 2 + par
    grp_sorted = grp[order]
    starts = np.zeros(NCORES * NW * 2 + 1, np.int64)
    np.add.at(starts, grp_sorted + 1, 1)
    gstart = np.cumsum(starts)[:-1]
    pos = (np.arange(E) - gstart[grp_sorted])[inv]
    srow = np.where(par == 0, pos, T_e * 128 + pos)

    gidx = np.zeros((NCORES, NW, 2, max(T_e, T_o) * 128), np.int16)
    gidx[core, wi, par, pos] = hidx.astype(np.int16)
    S = np.zeros((NCORES, NW, TT * 128, 128), np.float32)
    np.add.at(S, (core, wi, srow, slot_d), 1.0)

    # S blob for lhsT: sblob[c, p, (w*TT+t)*128 + d] = S[c, w, t*128+p, d]
    S5 = S.reshape(NCORES, NW, TT, 128, 128)
    sblob = np.ascontiguousarray(S5.transpose(0, 3, 1, 2, 4)).reshape(NCORES, 128, NW * TT * 128).astype(f8e4)

    # gidx blob [c, 128, cols]: per (w,par) segment; 16-row wrap replicated x8
    seg_e, seg_o = T_e * 8, T_o * 8
    cols = NW * (seg_e + seg_o)
    gblob = np.zeros((NCORES, 128, cols), np.int16)
    offs_e = np.arange(NW) * (seg_e + seg_o)
    offs_o = offs_e + seg_e
    for p_, (Tp, offs) in enumerate([(T_e, offs_e), (T_o, offs_o)]):
        seg = gidx[:, :, p_, :Tp * 128].reshape(NCORES, NW, Tp * 8, 16)  # [c,w,j,r]
        seg = seg.transpose(0, 3, 1, 2)                                   # [c,r(16),w,j]
        for w in range(NW):
            blk = np.tile(seg[:, :, w, :], (1, 8, 1))                     # [c,128,j]
            gblob[:, :, offs[w]:offs[w] + Tp * 8] = blk

    def to_sb(vec):
        v = np.zeros(NCORES * DPC, np.float32)
        v[:N] = vec[:N]
        return np.ascontiguousarray(v.reshape(NCORES, NW, 128).transpose(0, 2, 1))  # [c,128,NW]

    ad_sb = to_sb(alpha * dinv)
    a_sb = to_sb(a)
    dinv_sb = to_sb(dinv)
    bi = np.zeros((NCORES * DPC, DF), np.float32)
    bi[:N] = binit
    binit_sb = np.ascontiguousarray(
        bi.reshape(NCORES, NW, 128, DF).transpose(0, 2, 1, 3)).reshape(NCORES, 128, NW * DF)

    table0 = np.zeros((TROWS, 128), np.float32)
    xt0 = dinv[:, None] * out0
    for c in range(NCORES):
        lo, hi = DPC * c, min(DPC * (c + 1), N)
        table0[RR * c: RR * c + (hi - lo), :DF] = xt0[lo:hi]
        table0[RR * c + DPC, :DF] = out0[lo:hi].sum(axis=0)
    table0 = np.ascontiguousarray(table0.astype(bf16).reshape(HT, 256))

    return dict(T_e=T_e, T_o=T_o, alpha=alpha, table0=table0, sblob=sblob,
                gblob=gblob, ad=ad_sb, a=a_sb, dinv=dinv_sb, binit=binit_sb)


def _split_multiwaits(nc):
    """Walrus codegen only encodes one sync wait per TPB instruction; hoist
    extra waits onto preceding NoOps on the same engine."""
    for blk in nc.m.functions[0].blocks:
        insts = blk.instructions
        i = 0
        while i < len(insts):
            inst = insts[i]
            si = getattr(inst, "sync_info", None)
            if si is not None and len(si.on_wait) > 1:
                waits = list(si.on_wait)
                for k, wx in enumerate(waits[:-1]):
                    nop = mybir.InstNoOp(
                        name=f"{inst.name}-sw{k}",
                        sync_info=mybir.SyncInfo(on_wait=[wx], on_update=[]),
                        bass_nofuse=True,
                        engine=inst.engine,
                    )
                    nc.register_instruction(nop)
                    insts.insert(i, nop)
                    i += 1
                si.on_wait = [waits[-1]]
            i += 1


# ----------------------------------------------------------------- builder
def build_graph(T_e, T_o, alpha, n_iter, n_devices=NCORES, collectives=True, do_gather=True, do_pe=True):
    TT = T_e + T_o
    seg_e, seg_o = T_e * 8, T_o * 8
    gcols = NW * (seg_e + seg_o)
    dt = mybir.dt
    Copy = mybir.ActivationFunctionType.Copy

    nc = bacc.Bacc("TRN2", target_bir_lowering=False, debug=False, num_devices=n_devices)

    table0 = nc.declare_dram_parameter("table0", [HT, 256], dt.bfloat16, isOutput=False)
    sblob_d = nc.declare_dram_parameter("sblob", [128, NW * TT * 128], dt.float8e4, isOutput=False)
    gidx_d = nc.declare_dram_parameter("gidx", [128, gcols], dt.int16, isOutput=False)
    ad_d = nc.declare_dram_parameter("ad", [128, NW], dt.float32, isOutput=False)
    a_d = nc.declare_dram_parameter("a", [128, NW], dt.float32, isOutput=False)
    dinv_d = nc.declare_dram_parameter("dinv", [128, NW], dt.float32, isOutput=False)
    binit_d = nc.declare_dram_parameter("binit", [128, NW * DF], dt.float32, isOutput=False)
    out_d = nc.declare_dram_parameter("out", [128, NW * DF], dt.float32, isOutput=True)

    with tile.TileContext(nc) as tc:
        with (
            tc.tile_pool(name="const", bufs=1) as constp,
            tc.tile_pool(name="gpool", bufs=3) as gpool,
            tc.tile_pool(name="npool", bufs=3) as npool,
            tc.tile_pool(name="mpool", bufs=2) as mpool,
            tc.tile_pool(name="pp1", bufs=2, space="PSUM") as pp1,
            tc.tile_pool(name="pp2", bufs=2, space="PSUM") as pp2,
            tc.tile_pool(name="pp3", bufs=2, space="PSUM") as pp3,
            tc.tile_pool(name="dpool", bufs=2, space="DRAM") as dpool,
        ):
            s_sb = constp.tile([128, NW * TT * 128], dt.float8e4)
            gidx_sb = constp.tile([128, gcols], dt.int16)
            ad_sb = constp.tile([128, NW], dt.float32)
            a_sb = constp.tile([128, NW], dt.float32)
            dinv_sb = constp.tile([128, NW], dt.float32)
            binit_sb = constp.tile([128, NW * DF], dt.float32)
            ones_sb = constp.tile([128, 1], dt.float32)
            onesrow_sb = constp.tile([1, 128], dt.float32)
            xstage_sb = constp.tile([128, NW * 128], dt.bfloat16)
            xex_sb = constp.tile([1, 128], dt.bfloat16)

            nc.sync.dma_start(out=s_sb[:], in_=sblob_d[:])
            nc.sync.dma_start(out=gidx_sb[:], in_=gidx_d[:])
            nc.sync.dma_start(out=ad_sb[:], in_=ad_d[:])
            nc.sync.dma_start(out=a_sb[:], in_=a_d[:])
            nc.sync.dma_start(out=dinv_sb[:], in_=dinv_d[:])
            nc.sync.dma_start(out=binit_sb[:], in_=binit_d[:])
            nc.vector.memset(ones_sb[:], 1.0)
            nc.vector.memset(onesrow_sb[:], 1.0)
            nc.vector.memset(xstage_sb[:], 0.0)
            nc.vector.memset(xex_sb[:], 0.0)

            # one shared register for gather valid-counts (to_reg per call leaks
            # a Pool register and the file is ~64 deep)
            nreg_e = nc.gpsimd.to_reg(T_e * 128)
            nreg_o = nreg_e if T_o == T_e else nc.gpsimd.to_reg(T_o * 128)

            tbl = table0
            for it in range(n_iter):
                last = it == n_iter - 1

                # ---- mean from extras rows
                ex_sb = mpool.tile([1, 1024], dt.bfloat16, tag="ex")
                for c in range(NCORES):
                    nc.sync.dma_start(out=ex_sb[0:1, c * 128:(c + 1) * 128],
                                      in_=tbl[3140 * c + 3136, 0:128])
                e32 = mpool.tile([1, 1024], dt.float32, tag="e32")
                nc.vector.tensor_copy(e32[:], ex_sb[:])
                nc.vector.tensor_add(e32[0:1, 0:512], e32[0:1, 0:512], e32[0:1, 512:1024])
                nc.vector.tensor_add(e32[0:1, 0:256], e32[0:1, 0:256], e32[0:1, 256:512])
                nc.vector.tensor_add(e32[0:1, 0:128], e32[0:1, 0:128], e32[0:1, 128:256])
                meanbar = mpool.tile([1, 128], dt.float32, tag="mb")
                nc.scalar.activation(meanbar[:], e32[0:1, 0:128], Copy,
                                     scale=(1.0 - alpha) / N)

                ps2 = pp2.tile([128, DF], dt.float32, tag="ps2")
                nc.tensor.matmul(ps2[:], onesrow_sb[0:1, :], meanbar[0:1, 0:DF],
                                 start=True, stop=True, skip_group_check=True)
                ps3 = pp3.tile([1, DF], dt.float32, tag="ps3")
                for w in range(NW):
                    ge = gpool.tile([128, T_e, 128], dt.bfloat16, tag="ge")
                    go = gpool.tile([128, T_o, 128], dt.bfloat16, tag="go")
                    off = w * (seg_e + seg_o)
                    if do_gather:
                        nc.gpsimd.dma_gather(ge[:], tbl[:, 0:128],
                                             gidx_sb[:, off:off + seg_e],
                                             T_e * 128, nreg_e, 128, elem_step=256,
                                             single_packet=False)
                        nc.gpsimd.dma_gather(go[:], tbl[:, 128:256],
                                             gidx_sb[:, off + seg_e:off + seg_e + seg_o],
                                             T_o * 128, nreg_o, 128, elem_step=256,
                                             single_packet=False)
                    if not do_pe:
                        continue
                    ps1 = pp1.tile([128, DF], dt.float32, tag="ps1")
                    for t in range(TT):
                        rhs = ge[:, t, 0:DF] if t < T_e else go[:, t - T_e, 0:DF]
                        nc.tensor.matmul(ps1[:], s_sb[:, (w * TT + t) * 128:(w * TT + t + 1) * 128],
                                         rhs, start=(t == 0), stop=(t == TT - 1),
                                         skip_group_check=True)
                    newt = npool.tile([128, DF], dt.float32, tag="newt")
                    nc.scalar.activation(newt[:], ps1[:], Copy, scale=ad_sb[:, w:w + 1])
                    nc.vector.tensor_add(newt[:], newt[:], ps2[:])
                    nc.scalar.activation(newt[:], newt[:], Copy, scale=a_sb[:, w:w + 1])
                    nc.vector.tensor_add(newt[:], newt[:], binit_sb[:, w * DF:(w + 1) * DF])
                    if last:
                        nc.sync.dma_start(out=out_d[:, w * DF:(w + 1) * DF], in_=newt[:])
                    else:
                        nc.tensor.matmul(ps3[:], ones_sb[:, 0:1], newt[:],
                                         start=(w == 0), stop=(w == NW - 1),
                                         skip_group_check=True)
                        nc.scalar.activation(xstage_sb[:, w * 128:w * 128 + DF], newt[:],
                                             Copy, scale=dinv_sb[:, w:w + 1])

                if not last:
                    nc.scalar.activation(xex_sb[0:1, 0:DF], ps3[0:1, 0:DF], Copy)
                    stage = dpool.tile([RR * 128], dt.bfloat16, tag="stage")
                    nc.sync.dma_start(
                        out=stage[0:DPC * 128].rearrange("(w d f) -> d w f", w=NW, d=128, f=128),
                        in_=xstage_sb[:].rearrange("d (w f) -> d w f", w=NW, f=128))
                    nc.sync.dma_start(out=stage[DPC * 128:DPC * 128 + 128], in_=xex_sb[0:1, :])
                    ntbl = dpool.tile([HT, 256], dt.bfloat16, tag="table", addr_space="Shared")
                    if collectives:
                        nc.gpsimd.collective_compute(
                            "AllGather", mybir.AluOpType.bypass,
                            replica_groups=[list(range(NCORES))],
                            ins=[stage[:]],
                            outs=[ntbl[:].rearrange("a b -> (a b)")])
                    else:
                        nc.sync.dma_start(
                            out=ntbl[0:RR // 2, :].rearrange("a b -> (a b)"),
                            in_=stage[:])
                    tbl = ntbl
    nc.compile()
    return nc


# ----------------------------------------------------------------- entry
def run_full(inputs, trace=False, **spmd_kwargs):
    prep = host_prep(inputs["x"], inputs["edge_index"], inputs["known_idx"],
                     inputs["alpha"], inputs["beta"])
    nc = build_graph(prep["T_e"], prep["T_o"], prep["alpha"], N_ITER)
    in_maps = [
        dict(table0=prep["table0"], sblob=prep["sblob"][c], gidx=prep["gblob"][c],
             ad=prep["ad"][c], a=prep["a"][c], dinv=prep["dinv"][c],
             binit=prep["binit"][c])
        for c in range(NCORES)
    ]
    res = run_bass_kernel_spmd(nc, in_maps, core_ids=list(range(NCORES)),
                               trace=trace, **spmd_kwargs)
    outs = []
    for c in range(NCORES):
        o = np.asarray(res.results[c]["out"], np.float32)  # [128, NW*DF]
        outs.append(o.reshape(128, NW, DF).transpose(1, 0, 2).reshape(DPC, DF))
    return np.concatenate(outs)[:N].astype(np.float32), res


def kernel(**inputs) -> np.ndarray:
    out, _ = run_full(inputs)
    return out


if __name__ == "__main__":
    d = np.load("/tmp/inputs.npz")
    ins = {k: d[k] for k in d.files}
    got = kernel(**ins)
    print("kernel output", got.shape, float(np.linalg.norm(got)))

